# revision 36
# baseline (speedup 1.0000x reference)
"""Trainium2 Bass kernel for ControlLoRACrossAttnProcessor (final).

Batch x head-group sharding over 8 NeuronCores: core c owns batch c//4
and heads 2*(c%4), 2*(c%4)+1.  Each core computes both heads' full
attention over its batch and a partial output projection (contraction
over its 256 Wo rows); the host sums 4 partials per batch and adds bo.
All matmuls bf16 (PSUM fp32); measured ~153-155us vs the 174.7us
baseline, rel err 3.9e-3.

Schedule design (why it is fast):
- MM-granularity interleaving: independent projection / out-proj /
  LoRA matmuls are emitted *between* the score/PV matmul pairs of each
  attention phase via a fill queue, so the PE always has queued work
  while PV waits on ScalarE's exp (the per-p-step chain limiter).
- The softmax-denominator finish (ones-matmul + reciprocal + normalize)
  for (h, s) is deferred into the NEXT phase's fill stream, so the PE
  never stalls on the DVE pair-sum tree; the final strip normalizes in
  quarter-strips so its out-proj tiles unblock early.
- HAM warmup: two short *accumulating* matmul chains run back-to-back
  during the ~8us fixed DMA/preamble lead-in, flipping the PE clock
  gate to 2.4GHz before real data lands (spaced dummies never do --
  isolated MMs pace at ~0.5-1.2us and never fill a 3.4us busy window).
- Startup: first strip DMA'd in 8 chunks behind the first weights, so
  k/q projections start per-chunk; exp table preloaded with a dummy
  activation during the lead-in.
- Engine balance: pre-phase PSUM copies on ScE (idle before exp work
  starts), mid-kernel copies on DVE, 2 of 8 out-proj copies per strip
  on ScE; LoRA rides the A-phases where ScE/DVE have slack.

Measured dead ends (do not revisit without new evidence): GpSimd
tensor_add is 2.6us per [128,1024] tile; splitting exp into [128,512]
halves costs +257ns/pair in fixed per-op overhead; fp8 DoubleRow
out-projection hung the exec unit; fp8 on any two of {qkv, pv, out}
exceeds the 2e-2 error gate (each alone is 1.4-1.9e-2).
"""

import sys

for _p in ("/opt/trn_rl_repo", "/root/.axon_site"):
    if _p not in sys.path:
        sys.path.insert(0, _p)

import numpy as np
import ml_dtypes

import concourse.bass as bass  # noqa: E402
import concourse.mybir as mybir  # noqa: E402
import concourse.bass_isa as bass_isa  # noqa: E402
from concourse import bacc  # noqa: E402
from concourse.bass_utils import run_bass_kernel_spmd  # noqa: E402
from concourse.tile import TileContext  # noqa: E402
from concourse.masks import make_identity  # noqa: E402

dt = mybir.dt

B, S, D = 2, 2048, 1024
H = 8
HD = 128
RANK = 4
N_CORES = 8
SG = B * S
ROWS_PER_CORE = SG // N_CORES      # 512 lora rows per core
NSTRIP = S // 512                  # 4 query strips of 512
NKT = S // 128                     # 16 key tiles of 128
HEADS_PER_CORE = 2
INV_SQRT_HD = 1.0 / np.sqrt(np.float32(HD))

F32 = dt.float32
BF16 = dt.bfloat16
F8 = dt.float8e4

# fp8 out-projection scales: atn is written as 32*(softmax-avg of v)
# (bounded by 32*max|v| ~ 115 < 240), Wo is host-scaled by 1024; the
# host divides the partial sums by 32*1024 when reducing.
S_ATN = 32.0
S_WO = 1024.0

# offload the p3/p5 softmax pair-sums to GpSimd (DVE relief in the
# out-projection phases).  Measured 2.6us per [128,1024] add on HW --
# slower than DVE and it stalls the accumulate chain, so OFF.
GP_PAIR = False

_CACHE = {}


class Fill:
    """FIFO of (pe_cost_ns, emit_fn) quanta pulled between attention
    matmul groups so the PE queue never starves while PV waits on exp."""

    def __init__(self):
        self.q = []

    def add(self, cost, fn):
        self.q.append((cost, fn))

    def pull(self, budget):
        spent = 0
        while self.q and spent < budget:
            c, fn = self.q.pop(0)
            fn()
            spent += c

    def drain(self):
        while self.q:
            _, fn = self.q.pop(0)
            fn()


def build_program():
    if "nc" in _CACHE:
        return _CACHE["nc"]

    nc = bacc.Bacc("TRN2", target_bir_lowering=False, debug=False,
                   num_devices=N_CORES)

    # Pre-shuffled host layouts: every DMA is a plain contiguous 2D copy.
    xsh = nc.declare_dram_parameter("xsh", [NSTRIP * 128, 8 * 512], BF16,
                                    isOutput=False)
    wqT = nc.declare_dram_parameter("wqT", [2 * 128, 8 * HD], BF16,
                                    isOutput=False)
    wkT = nc.declare_dram_parameter("wkT", [2 * 128, 8 * HD], BF16,
                                    isOutput=False)
    wvT = nc.declare_dram_parameter("wvT", [2 * 128, 8 * HD], BF16,
                                    isOutput=False)
    woT = nc.declare_dram_parameter("woT", [256, D], BF16, isOutput=False)
    cT = nc.declare_dram_parameter("cT", [128, 8 * ROWS_PER_CORE], BF16,
                                   isOutput=False)
    ldT = nc.declare_dram_parameter("ldT", [128, 8 * RANK], BF16,
                                    isOutput=False)
    luT = nc.declare_dram_parameter("luT", [RANK, D], BF16, isOutput=False)
    out = nc.declare_dram_parameter("out", [S, D], BF16, isOutput=True)
    lora_out = nc.declare_dram_parameter("lora_out", [128, 4 * D],
                                         BF16, isOutput=True)

    with TileContext(nc) as tc:
        with tc.tile_pool(name="const", bufs=1) as constp, \
             tc.tile_pool(name="wts", bufs=1) as wts, \
             tc.tile_pool(name="xs", bufs=1) as xsp, \
             tc.tile_pool(name="qkv", bufs=1) as qkvp, \
             tc.tile_pool(name="stage", bufs=2) as stagep, \
             tc.tile_pool(name="es", bufs=6) as esp, \
             tc.tile_pool(name="rp", bufs=2) as rp, \
             tc.tile_pool(name="pp", bufs=2) as pp, \
             tc.tile_pool(name="small", bufs=2) as smallp, \
             tc.tile_pool(name="outp", bufs=2) as outp, \
             tc.tile_pool(name="sc_ps", bufs=2, space="PSUM") as sc_ps, \
             tc.tile_pool(name="at_ps", bufs=2, space="PSUM") as at_ps, \
             tc.tile_pool(name="op_ps", bufs=2, space="PSUM") as op_ps:

            mm = nc.tensor.matmul

            # ---- constants ----
            ident = constp.tile([128, 128], BF16, tag="ident")
            make_identity(nc, ident[:])
            dummy = constp.tile([128, 512], BF16, tag="dummy")
            nc.vector.memset(dummy[:], 0.0)
            ones_sq = constp.tile([128, 128], BF16, tag="ones_sq")
            nc.vector.memset(ones_sq[:], 1.0)

            # ---- DMA loads, ordered for earliest compute start ----
            ld_sb = wts.tile([128, 8 * RANK], BF16, tag="ld")
            lu_sb = wts.tile([RANK, D], BF16, tag="lu")
            ct_sb = wts.tile([128, 8 * ROWS_PER_CORE], BF16, tag="ct")

            w_sb = {}
            for h in range(HEADS_PER_CORE):
                for nm in ("q", "k", "v"):
                    w_sb[(nm, h)] = wts.tile([128, 8 * HD], BF16,
                                             tag=f"w{nm}{h}", name=f"w{nm}{h}")
            wo_sb = [wts.tile([HD, D], BF16, tag=f"wo{h}", name=f"wo{h}")
                     for h in range(HEADS_PER_CORE)]
            xs = [xsp.tile([128, 8 * 512], BF16, tag=f"xs{s}", name=f"xs{s}")
                  for s in range(NSTRIP)]

            wsrc = {"q": wqT, "k": wkT, "v": wvT}

            def load_w(nm, h):
                nc.sync.dma_start(out=w_sb[(nm, h)][:],
                                  in_=wsrc[nm][h * 128:(h + 1) * 128, :])

            # First-wave DMAs ride the GpSimd/Vector descriptor queues:
            # those engines clear their preambles ~3us before SyncE issues
            # its first descriptor, so wk0/wq0/xs0 start moving earlier
            # AND SyncE's serial descriptor stream begins directly with
            # the later strips.
            nc.gpsimd.dma_start(out=w_sb[("k", 0)][:],
                                in_=wkT[0:128, :])
            nc.scalar.dma_start(out=w_sb[("q", 0)][:],
                                in_=wqT[0:128, :])
            for ch in range(8):
                eng = nc.gpsimd if ch < 4 else nc.scalar
                eng.dma_start(
                    out=xs[0][:, ch * 512:(ch + 1) * 512],
                    in_=xsh[0:128, ch * 512:(ch + 1) * 512])
            load_w("v", 0)
            nc.scalar.dma_start(out=ld_sb[:], in_=ldT[:])
            nc.scalar.dma_start(out=lu_sb[:], in_=luT[:])
            for s in range(1, NSTRIP):
                for half in range(2):
                    nc.sync.dma_start(
                        out=xs[s][:, half * 2048:(half + 1) * 2048],
                        in_=xsh[s * 128:(s + 1) * 128,
                                half * 2048:(half + 1) * 2048])
            for nm in ("k", "q", "v"):
                load_w(nm, 1)
            for h in range(HEADS_PER_CORE):
                nc.sync.dma_start(out=wo_sb[h][:],
                                  in_=woT[h * HD:(h + 1) * HD, :])
            nc.sync.dma_start(out=ct_sb[:], in_=cT[:])

            # ---- PE warmup + ACT exp-table preload during DMA lead-in.
            # Accumulating matmul chains have no inter-MM dependencies, so
            # they run back-to-back: a contiguous >3.4us busy window that
            # flips the HAM clock gate to 2.4GHz before real data lands.
            for grp, n_mm in ((0, 8), (1, 2)):
                wu_ps = op_ps.tile([128, 512], F32, tag="op", name="wu_ps")
                for i in range(n_mm):
                    mm(wu_ps[:], dummy[:, 0:128], dummy[:],
                       start=(i == 0), stop=(i == n_mm - 1))
            exp_warm = smallp.tile([128, 16], BF16, tag="spf",
                                   name="exp_warm")
            nc.scalar.activation(exp_warm[:], dummy[:, 0:16],
                                 mybir.ActivationFunctionType.Exp)

            # ---- persistent SBUF state ----
            qt = [qkvp.tile([HD, S], BF16, tag=f"qt{h}", name=f"qt{h}")
                  for h in range(HEADS_PER_CORE)]
            kt = [qkvp.tile([HD, S], BF16, tag=f"kt{h}", name=f"kt{h}")
                  for h in range(HEADS_PER_CORE)]
            v_sb = [qkvp.tile([128, S], BF16, tag=f"v{h}", name=f"v{h}")
                    for h in range(HEADS_PER_CORE)]
            atn = [qkvp.tile([HD, S], BF16, tag=f"atn{h}", name=f"atn{h}")
                   for h in range(HEADS_PER_CORE)]

            # ---- projection emitters ----
            def proj_mms(st, nm, h, s, d0, d1):
                if d0 == 0:
                    st["ps"] = op_ps.tile([128, 512], F32, tag="op",
                                          name=f"pj_{nm}{h}{s}")
                for d in range(d0, d1):
                    mm(st["ps"][:],
                       w_sb[(nm, h)][:, d * HD:(d + 1) * HD],
                       xs[s][:, d * 512:(d + 1) * 512],
                       start=(d == 0), stop=(d == 7),
                       skip_group_check=True)

            def qk_quanta(nm, h, s, ce):
                st = {}
                dst = qt[h] if nm == "q" else kt[h]
                sl = slice(s * 512, (s + 1) * 512)
                return [
                    (900, lambda: proj_mms(st, nm, h, s, 0, 4)),
                    (900, lambda: proj_mms(st, nm, h, s, 4, 8)),
                    (0, lambda: ce(dst[:, sl], st["ps"][:])),
                ]

            def v_quanta(h, s, ce):
                st = {}
                sl = slice(s * 512, (s + 1) * 512)

                def stage_copy():
                    st["vt"] = stagep.tile([HD, 512], BF16, tag="vst",
                                           name="vt_stage")
                    ce(st["vt"][:], st["ps"][:])

                def transp():
                    st["tq"] = op_ps.tile([128, 512], BF16, tag="op",
                                          name="tq")
                    for i in range(4):
                        nc.tensor.transpose(
                            st["tq"][:, i * 128:(i + 1) * 128],
                            st["vt"][:, i * 128:(i + 1) * 128], ident[:])

                return [
                    (900, lambda: proj_mms(st, "v", h, s, 0, 4)),
                    (900, lambda: proj_mms(st, "v", h, s, 4, 8)),
                    (0, stage_copy),
                    (500, transp),
                    (0, lambda: ce(v_sb[h][:, sl], st["tq"][:])),
                ]

            def qkv_quanta(h, s, ce):
                return (qk_quanta("k", h, s, ce) + v_quanta(h, s, ce)
                        + qk_quanta("q", h, s, ce))

            # ---- out-projection quanta for one strip ----
            def outp_quanta(s):
                quanta = []
                for j in range(4):
                    j_even = (j % 2 == 0)
                    st = {}
                    c_sl = slice(s * 512 + j * 128, s * 512 + (j + 1) * 128)
                    r0 = s * 512 + j * 128

                    def mms(st=st, c_sl=c_sl):
                        st["ps"] = []
                        for g in range(2):
                            ps = op_ps.tile([128, 512], F32, tag="op",
                                            name="opj_ps")
                            for h in range(HEADS_PER_CORE):
                                mm(ps[:], atn[h][:, c_sl],
                                   wo_sb[h][:, g * 512:(g + 1) * 512],
                                   start=(h == 0), stop=(h == 1),
                                   skip_group_check=True)
                            st["ps"].append(ps)

                    def fin(st=st, r0=r0, j_even=j_even):
                        o_sb = outp.tile([128, D], BF16, tag="osb",
                                         name="o_sb")
                        # 2 of 8 PSUM->SBUF copies per strip go to ScE so
                        # DVE isn't the out-proj phase bottleneck
                        if j_even:
                            nc.scalar.copy(o_sb[:, 0:512], st["ps"][0][:])
                        else:
                            nc.vector.tensor_copy(o_sb[:, 0:512],
                                                  st["ps"][0][:])
                        nc.vector.tensor_copy(o_sb[:, 512:1024],
                                              st["ps"][1][:])
                        nc.sync.dma_start(out=out[r0:r0 + 128, :],
                                          in_=o_sb[:])

                    quanta.append((900, mms))
                    quanta.append((0, fin))
                return quanta

            # ---- LoRA quanta ----
            lora_st = {}

            def lora_down(d0, d1):
                if d0 == 0:
                    lora_st["dn_ps"] = op_ps.tile([128, ROWS_PER_CORE], F32,
                                                  tag="op", name="dn_ps")
                for d in range(d0, d1):
                    mm(lora_st["dn_ps"][0:RANK, :],
                       ld_sb[:, d * RANK:(d + 1) * RANK],
                       ct_sb[:, d * ROWS_PER_CORE:(d + 1) * ROWS_PER_CORE],
                       start=(d == 0), stop=(d == 7),
                       skip_group_check=True)

            def lora_dn_copy():
                dn_sb = smallp.tile([RANK, ROWS_PER_CORE], BF16, tag="dn",
                                    name="dn_sb", bufs=1)
                nc.vector.tensor_copy(dn_sb[:], lora_st["dn_ps"][0:RANK, :])
                lora_st["dn_sb"] = dn_sb
                lora_st["lo_sb"] = outp.tile([128, 4 * D], BF16, tag="lo",
                                             bufs=1, name="lo_sb")

            def lora_up(j):
                for g in range(2):
                    up_ps = op_ps.tile([128, 512], F32, tag="op",
                                       name="up_ps")
                    mm(up_ps[:],
                       lora_st["dn_sb"][:, j * 128:(j + 1) * 128],
                       lu_sb[:, g * 512:(g + 1) * 512],
                       start=True, stop=True, skip_group_check=True)
                    nc.vector.tensor_copy(
                        lora_st["lo_sb"][:, j * D + g * 512:
                                         j * D + (g + 1) * 512],
                        up_ps[:])

            def lora_dma():
                nc.sync.dma_start(out=lora_out[:], in_=lora_st["lo_sb"][:])

            lora_quanta = (
                [(900, lambda: lora_down(0, 4)),
                 (900, lambda: lora_down(4, 8)),
                 (0, lora_dn_copy)]
                + [(500, lambda j=j: lora_up(j)) for j in range(4)]
                + [(0, lora_dma)])

            # ---- attention phase (scores + exp + PV + pair-sum tree),
            #      with fill quanta interleaved between matmul groups ----
            add = nc.vector.tensor_add

            def attn_phase(h, s, fill):
                q_sl = slice(s * 512, (s + 1) * 512)
                at = at_ps.tile([HD, 512], F32, tag="at", name="at")
                r = rp.tile([128, 1024], BF16, tag="r", name="r")
                prev_es = None
                for p in range(8):
                    scp = sc_ps.tile([128, 1024], F32, tag="sc", name="scp")
                    for i in range(2):
                        ktile = 2 * p + i
                        mm(scp[:, i * 512:(i + 1) * 512],
                           kt[h][:, ktile * 128:(ktile + 1) * 128],
                           qt[h][:, q_sl],
                           start=True, stop=True, skip_group_check=True)
                    if p == 0:
                        fill.pull(1500)
                    es_p = esp.tile([128, 1024], BF16, tag="es",
                                    name="es_p")
                    nc.scalar.activation(
                        es_p[:], scp[:], mybir.ActivationFunctionType.Exp,
                        scale=float(INV_SQRT_HD))
                    for i in range(2):
                        ktile = 2 * p + i
                        mm(at[:],
                           v_sb[h][:, ktile * 128:(ktile + 1) * 128],
                           es_p[:, i * 512:(i + 1) * 512],
                           start=(ktile == 0), stop=(ktile == NKT - 1),
                           skip_group_check=True)
                    if p % 2 == 1:
                        if p == 1:
                            add(r[:], prev_es[:], es_p[:])
                        else:
                            t = pp.tile([128, 1024], BF16, tag="pp",
                                        name="tpair")
                            # p3/p5 pair-sums ride the otherwise-idle
                            # GpSimd so DVE keeps up in the out-proj
                            # phases; p7 stays on DVE (shortest tail).
                            if GP_PAIR and p in (3, 5):
                                nc.gpsimd.tensor_add(t[:], prev_es[:],
                                                     es_p[:])
                            else:
                                add(t[:], prev_es[:], es_p[:])
                            add(r[:], r[:], t[:])
                    prev_es = es_p
                    if p < 7:
                        fill.pull(900)
                spf = smallp.tile([128, 512], BF16, tag="sp2", name="spf")
                add(spf[:], r[:, 0:512], r[:, 512:1024])

                def finish(split_mul=False):
                    den = op_ps.tile([128, 512], F32, tag="op",
                                     name="den_bc")
                    mm(den[:], ones_sq[:], spf[:],
                       start=True, stop=True, skip_group_check=True)
                    rc = smallp.tile([128, 512], F32, tag="rc", name="rc")
                    nc.vector.reciprocal_approx_fast(rc[:], den[:])
                    if split_mul:
                        # quarter-strip normalize so each out-proj j-tile
                        # of the final strip unblocks as early as possible
                        for j in range(4):
                            jj = slice(j * 128, (j + 1) * 128)
                            nc.vector.tensor_mul(
                                atn[h][:, s * 512 + j * 128:
                                       s * 512 + (j + 1) * 128],
                                at[:, jj], rc[:, jj])
                    else:
                        nc.vector.tensor_mul(atn[h][:, q_sl],
                                             at[:], rc[:])
                return finish

            # ---- schedule ----
            fill = Fill()
            ce_s = nc.scalar.copy
            ce_v = nc.vector.tensor_copy

            # pre-phase: h0 QKV, strip-availability order, copies on ScE
            pre = []
            pre += qk_quanta("k", 0, 0, ce_s)
            pre += qk_quanta("q", 0, 0, ce_s)
            pre += v_quanta(0, 0, ce_s)
            for s in range(1, NSTRIP):
                pre += qk_quanta("k", 0, s, ce_s)
                pre += v_quanta(0, s, ce_s)
            for _, fn in pre:
                fn()

            # A phases: attn(h0, s) filled with remaining h0 q-projs,
            # h1 qkv, lora (ScE/DVE have slack here), deferred finishes.
            for q in [0, 1, 2]:
                for c, fn in qk_quanta("q", 0, 1 + q, ce_v):
                    fill.add(c, fn)
            fin = {}
            fin[(0, 0)] = attn_phase(0, 0, fill)

            a_extra = {1: lora_quanta[0:3], 2: lora_quanta[3:5],
                       3: lora_quanta[5:]}
            for s in range(1, NSTRIP):
                for c, fn in qkv_quanta(1, s - 1, ce_v):
                    fill.add(c, fn)
                fill.add(250, fin[(0, s - 1)])
                for c, fn in a_extra[s]:
                    fill.add(c, fn)
                fin[(0, s)] = attn_phase(0, s, fill)

            # B phases: attn(h1, s) filled with last h1 qkv,
            # out-projections, and deferred finishes.
            for c, fn in qkv_quanta(1, 3, ce_v):
                fill.add(c, fn)
            fill.add(250, fin[(0, 3)])
            fin[(1, 0)] = attn_phase(1, 0, fill)

            op_q = {s: outp_quanta(s) for s in range(NSTRIP)}
            for s in range(3):
                fill.add(250, fin[(1, s)])
                for c, fn in op_q[s]:
                    fill.add(c, fn)
                fin[(1, s + 1)] = attn_phase(1, s + 1, fill)

            fill.add(250, lambda: fin[(1, 3)](split_mul=True))
            for c, fn in op_q[3]:
                fill.add(c, fn)
            fill.drain()

    nc.compile()
    _CACHE["nc"] = nc
    return nc


def _prep_in_maps(inputs):
    bf = ml_dtypes.bfloat16
    hidden = np.asarray(inputs["hidden_states"], dtype=np.float32)
    control = np.asarray(inputs["control_states"], dtype=np.float32)
    Wq = np.asarray(inputs["Wq"], dtype=np.float32)
    Wk = np.asarray(inputs["Wk"], dtype=np.float32)
    Wv = np.asarray(inputs["Wv"], dtype=np.float32)
    Wo = np.asarray(inputs["Wo"], dtype=np.float32)
    ldT = np.ascontiguousarray(
        np.asarray(inputs["lora_down"], dtype=np.float32).T.astype(bf)
        .reshape(8, 128, RANK).transpose(1, 0, 2).reshape(128, 8 * RANK))
    luT = np.ascontiguousarray(
        np.asarray(inputs["lora_up"], dtype=np.float32).T.astype(bf))

    def wshuf(wT):  # [1024, 256] -> [2*128, 8*128] (head-major rows)
        return np.ascontiguousarray(
            wT.reshape(8, 128, 2, HD).transpose(2, 1, 0, 3)
            .reshape(2 * 128, 8 * HD))

    xsh_b = []
    for b in range(B):
        xT = hidden[b].T.astype(bf)  # [1024, 2048]
        xsh_b.append(np.ascontiguousarray(
            xT.reshape(8, 128, NSTRIP, 512).transpose(2, 1, 0, 3)
            .reshape(NSTRIP * 128, 8 * 512)))
    cT_full = control.reshape(SG, D).T.astype(bf)

    in_maps = []
    for c in range(N_CORES):
        b = c // 4
        g = c % 4
        hs = slice(g * 256, (g + 1) * 256)
        rs = slice(c * ROWS_PER_CORE, (c + 1) * ROWS_PER_CORE)
        ct = cT_full[:, rs]  # [1024, 512]
        in_maps.append({
            "xsh": xsh_b[b],
            "wqT": wshuf(Wq[hs, :].T.astype(bf)),
            "wkT": wshuf(Wk[hs, :].T.astype(bf)),
            "wvT": wshuf(Wv[hs, :].T.astype(bf)),
            "woT": np.ascontiguousarray(Wo[:, hs].T.astype(bf)),
            "cT": np.ascontiguousarray(
                ct.reshape(8, 128, ROWS_PER_CORE).transpose(1, 0, 2)
                .reshape(128, 8 * ROWS_PER_CORE)),
            "ldT": ldT,
            "luT": luT,
        })
    return in_maps


def _reduce_outputs(results, bo):
    total = np.zeros((B, S, D), dtype=np.float32)
    for c in range(N_CORES):
        b = c // 4
        total[b] += results[c]["out"].astype(np.float32)
    total += np.asarray(bo, dtype=np.float32)[None, None, :]
    flat = total.reshape(SG, D)
    for c in range(N_CORES):
        rs = slice(c * ROWS_PER_CORE, (c + 1) * ROWS_PER_CORE)
        lo = results[c]["lora_out"].astype(np.float32)
        flat[rs] += lo.reshape(128, 4, D).transpose(1, 0, 2).reshape(
            ROWS_PER_CORE, D)
    return flat.reshape(B, S, D)


def kernel(**inputs):
    nc = build_program()
    in_maps = _prep_in_maps(inputs)
    res = run_bass_kernel_spmd(nc, in_maps, list(range(N_CORES)))
    return _reduce_outputs(res.results, inputs["bo"])


# revision 37
# speedup vs baseline: 1.0142x; 1.0142x over previous
"""Trainium2 Bass kernel for ControlLoRACrossAttnProcessor (final).

Batch x head-group sharding over 8 NeuronCores: core c owns batch c//4
and heads 2*(c%4), 2*(c%4)+1.  Each core computes both heads' full
attention over its batch and a partial output projection (contraction
over its 256 Wo rows); the host sums 4 partials per batch and adds bo.
All matmuls bf16 (PSUM fp32); measured ~153-155us vs the 174.7us
baseline, rel err 3.9e-3.

Schedule design (why it is fast):
- MM-granularity interleaving: independent projection / out-proj /
  LoRA matmuls are emitted *between* the score/PV matmul pairs of each
  attention phase via a fill queue, so the PE always has queued work
  while PV waits on ScalarE's exp (the per-p-step chain limiter).
- The softmax-denominator finish (ones-matmul + reciprocal + normalize)
  for (h, s) is deferred into the NEXT phase's fill stream, so the PE
  never stalls on the DVE pair-sum tree; the final strip normalizes in
  quarter-strips so its out-proj tiles unblock early.
- HAM warmup: two short *accumulating* matmul chains run back-to-back
  during the ~8us fixed DMA/preamble lead-in, flipping the PE clock
  gate to 2.4GHz before real data lands (spaced dummies never do --
  isolated MMs pace at ~0.5-1.2us and never fill a 3.4us busy window).
- Startup: first strip DMA'd in 8 chunks behind the first weights, so
  k/q projections start per-chunk; exp table preloaded with a dummy
  activation during the lead-in.
- Engine balance: pre-phase PSUM copies on ScE (idle before exp work
  starts), mid-kernel copies on DVE, 2 of 8 out-proj copies per strip
  on ScE; LoRA rides the A-phases where ScE/DVE have slack.

Measured dead ends (do not revisit without new evidence): GpSimd
tensor_add is 2.6us per [128,1024] tile; splitting exp into [128,512]
halves costs +257ns/pair in fixed per-op overhead; fp8 DoubleRow
out-projection hung the exec unit; fp8 on any two of {qkv, pv, out}
exceeds the 2e-2 error gate (each alone is 1.4-1.9e-2).
"""

import sys

for _p in ("/opt/trn_rl_repo", "/root/.axon_site"):
    if _p not in sys.path:
        sys.path.insert(0, _p)

import numpy as np
import ml_dtypes

import concourse.bass as bass  # noqa: E402
import concourse.mybir as mybir  # noqa: E402
import concourse.bass_isa as bass_isa  # noqa: E402
from concourse import bacc  # noqa: E402
from concourse.bass_utils import run_bass_kernel_spmd  # noqa: E402
from concourse.tile import TileContext  # noqa: E402
from concourse.masks import make_identity  # noqa: E402

dt = mybir.dt

B, S, D = 2, 2048, 1024
H = 8
HD = 128
RANK = 4
N_CORES = 8
SG = B * S
ROWS_PER_CORE = SG // N_CORES      # 512 lora rows per core
NSTRIP = S // 512                  # 4 query strips of 512
NKT = S // 128                     # 16 key tiles of 128
HEADS_PER_CORE = 2
INV_SQRT_HD = 1.0 / np.sqrt(np.float32(HD))

F32 = dt.float32
BF16 = dt.bfloat16
F8 = dt.float8e4

# fp8 out-projection scales: atn is written as 32*(softmax-avg of v)
# (bounded by 32*max|v| ~ 115 < 240), Wo is host-scaled by 1024; the
# host divides the partial sums by 32*1024 when reducing.
S_ATN = 32.0
S_WO = 1024.0

# offload the p3/p5 softmax pair-sums to GpSimd (DVE relief in the
# out-projection phases).  Measured 2.6us per [128,1024] add on HW --
# slower than DVE and it stalls the accumulate chain, so OFF.
GP_PAIR = False

_CACHE = {}


class Fill:
    """FIFO of (pe_cost_ns, emit_fn) quanta pulled between attention
    matmul groups so the PE queue never starves while PV waits on exp."""

    def __init__(self):
        self.q = []

    def add(self, cost, fn):
        self.q.append((cost, fn))

    def pull(self, budget):
        spent = 0
        while self.q and spent < budget:
            c, fn = self.q.pop(0)
            fn()
            spent += c

    def drain(self):
        while self.q:
            _, fn = self.q.pop(0)
            fn()


def build_program():
    if "nc" in _CACHE:
        return _CACHE["nc"]

    nc = bacc.Bacc("TRN2", target_bir_lowering=False, debug=False,
                   num_devices=N_CORES)

    # Pre-shuffled host layouts: every DMA is a plain contiguous 2D copy.
    xsh = nc.declare_dram_parameter("xsh", [NSTRIP * 128, 8 * 512], BF16,
                                    isOutput=False)
    wqT = nc.declare_dram_parameter("wqT", [2 * 128, 8 * HD], BF16,
                                    isOutput=False)
    wkT = nc.declare_dram_parameter("wkT", [2 * 128, 8 * HD], BF16,
                                    isOutput=False)
    wvT = nc.declare_dram_parameter("wvT", [2 * 128, 8 * HD], BF16,
                                    isOutput=False)
    woT = nc.declare_dram_parameter("woT", [256, D], BF16, isOutput=False)
    cT = nc.declare_dram_parameter("cT", [128, 8 * ROWS_PER_CORE], BF16,
                                   isOutput=False)
    ldT = nc.declare_dram_parameter("ldT", [128, 8 * RANK], BF16,
                                    isOutput=False)
    luT = nc.declare_dram_parameter("luT", [RANK, D], BF16, isOutput=False)
    out = nc.declare_dram_parameter("out", [S, D], BF16, isOutput=True)
    lora_out = nc.declare_dram_parameter("lora_out", [128, 4 * D],
                                         BF16, isOutput=True)

    with TileContext(nc) as tc:
        with tc.tile_pool(name="const", bufs=1) as constp, \
             tc.tile_pool(name="wts", bufs=1) as wts, \
             tc.tile_pool(name="xs", bufs=1) as xsp, \
             tc.tile_pool(name="qkv", bufs=1) as qkvp, \
             tc.tile_pool(name="stage", bufs=2) as stagep, \
             tc.tile_pool(name="es", bufs=6) as esp, \
             tc.tile_pool(name="rp", bufs=2) as rp, \
             tc.tile_pool(name="pp", bufs=2) as pp, \
             tc.tile_pool(name="small", bufs=2) as smallp, \
             tc.tile_pool(name="outp", bufs=2) as outp, \
             tc.tile_pool(name="sc_ps", bufs=2, space="PSUM") as sc_ps, \
             tc.tile_pool(name="at_ps", bufs=2, space="PSUM") as at_ps, \
             tc.tile_pool(name="op_ps", bufs=2, space="PSUM") as op_ps:

            mm = nc.tensor.matmul

            # ---- constants ----
            ident = constp.tile([128, 128], BF16, tag="ident")
            make_identity(nc, ident[:])
            dummy = constp.tile([128, 512], BF16, tag="dummy")
            nc.vector.memset(dummy[:], 0.0)
            ones_sq = constp.tile([128, 128], BF16, tag="ones_sq")
            nc.vector.memset(ones_sq[:], 1.0)

            # ---- DMA loads, ordered for earliest compute start ----
            ld_sb = wts.tile([128, 8 * RANK], BF16, tag="ld")
            lu_sb = wts.tile([RANK, D], BF16, tag="lu")
            ct_sb = wts.tile([128, 8 * ROWS_PER_CORE], BF16, tag="ct")

            w_sb = {}
            for h in range(HEADS_PER_CORE):
                for nm in ("q", "k", "v"):
                    w_sb[(nm, h)] = wts.tile([128, 8 * HD], BF16,
                                             tag=f"w{nm}{h}", name=f"w{nm}{h}")
            wo_sb = [wts.tile([HD, D], BF16, tag=f"wo{h}", name=f"wo{h}")
                     for h in range(HEADS_PER_CORE)]
            xs = [xsp.tile([128, 8 * 512], BF16, tag=f"xs{s}", name=f"xs{s}")
                  for s in range(NSTRIP)]

            wsrc = {"q": wqT, "k": wkT, "v": wvT}

            def load_w(nm, h):
                nc.sync.dma_start(out=w_sb[(nm, h)][:],
                                  in_=wsrc[nm][h * 128:(h + 1) * 128, :])

            # The two first-needed weight DMAs ride the GpSimd DGE queue
            # (its preamble clears ~3us before SyncE issues descriptors),
            # while the xs0 chunks lead SyncE's fast serial descriptor
            # stream -- first projection data is in SBUF by ~9us.
            nc.gpsimd.dma_start(out=w_sb[("k", 0)][:],
                                in_=wkT[0:128, :])
            nc.gpsimd.dma_start(out=w_sb[("q", 0)][:],
                                in_=wqT[0:128, :])
            for ch in range(8):
                nc.sync.dma_start(
                    out=xs[0][:, ch * 512:(ch + 1) * 512],
                    in_=xsh[0:128, ch * 512:(ch + 1) * 512])
            load_w("v", 0)
            nc.sync.dma_start(out=ld_sb[:], in_=ldT[:])
            nc.sync.dma_start(out=lu_sb[:], in_=luT[:])
            for s in range(1, NSTRIP):
                for half in range(2):
                    nc.sync.dma_start(
                        out=xs[s][:, half * 2048:(half + 1) * 2048],
                        in_=xsh[s * 128:(s + 1) * 128,
                                half * 2048:(half + 1) * 2048])
            for nm in ("k", "q", "v"):
                load_w(nm, 1)
            for h in range(HEADS_PER_CORE):
                nc.sync.dma_start(out=wo_sb[h][:],
                                  in_=woT[h * HD:(h + 1) * HD, :])
            nc.sync.dma_start(out=ct_sb[:], in_=cT[:])

            # ---- PE warmup + ACT exp-table preload during DMA lead-in.
            # Accumulating matmul chains have no inter-MM dependencies, so
            # they run back-to-back: a contiguous >3.4us busy window that
            # flips the HAM clock gate to 2.4GHz before real data lands.
            for grp, n_mm in ((0, 8), (1, 2)):
                wu_ps = op_ps.tile([128, 512], F32, tag="op", name="wu_ps")
                for i in range(n_mm):
                    mm(wu_ps[:], dummy[:, 0:128], dummy[:],
                       start=(i == 0), stop=(i == n_mm - 1))
            exp_warm = smallp.tile([128, 16], BF16, tag="spf",
                                   name="exp_warm")
            nc.scalar.activation(exp_warm[:], dummy[:, 0:16],
                                 mybir.ActivationFunctionType.Exp)

            # ---- persistent SBUF state ----
            qt = [qkvp.tile([HD, S], BF16, tag=f"qt{h}", name=f"qt{h}")
                  for h in range(HEADS_PER_CORE)]
            kt = [qkvp.tile([HD, S], BF16, tag=f"kt{h}", name=f"kt{h}")
                  for h in range(HEADS_PER_CORE)]
            v_sb = [qkvp.tile([128, S], BF16, tag=f"v{h}", name=f"v{h}")
                    for h in range(HEADS_PER_CORE)]
            atn = [qkvp.tile([HD, S], BF16, tag=f"atn{h}", name=f"atn{h}")
                   for h in range(HEADS_PER_CORE)]

            # ---- projection emitters ----
            def proj_mms(st, nm, h, s, d0, d1):
                if d0 == 0:
                    st["ps"] = op_ps.tile([128, 512], F32, tag="op",
                                          name=f"pj_{nm}{h}{s}")
                for d in range(d0, d1):
                    mm(st["ps"][:],
                       w_sb[(nm, h)][:, d * HD:(d + 1) * HD],
                       xs[s][:, d * 512:(d + 1) * 512],
                       start=(d == 0), stop=(d == 7),
                       skip_group_check=True)

            def qk_quanta(nm, h, s, ce):
                st = {}
                dst = qt[h] if nm == "q" else kt[h]
                sl = slice(s * 512, (s + 1) * 512)
                return [
                    (900, lambda: proj_mms(st, nm, h, s, 0, 4)),
                    (900, lambda: proj_mms(st, nm, h, s, 4, 8)),
                    (0, lambda: ce(dst[:, sl], st["ps"][:])),
                ]

            def v_quanta(h, s, ce):
                st = {}
                sl = slice(s * 512, (s + 1) * 512)

                def stage_copy():
                    st["vt"] = stagep.tile([HD, 512], BF16, tag="vst",
                                           name="vt_stage")
                    ce(st["vt"][:], st["ps"][:])

                def transp():
                    st["tq"] = op_ps.tile([128, 512], BF16, tag="op",
                                          name="tq")
                    for i in range(4):
                        nc.tensor.transpose(
                            st["tq"][:, i * 128:(i + 1) * 128],
                            st["vt"][:, i * 128:(i + 1) * 128], ident[:])

                return [
                    (900, lambda: proj_mms(st, "v", h, s, 0, 4)),
                    (900, lambda: proj_mms(st, "v", h, s, 4, 8)),
                    (0, stage_copy),
                    (500, transp),
                    (0, lambda: ce(v_sb[h][:, sl], st["tq"][:])),
                ]

            def qkv_quanta(h, s, ce):
                return (qk_quanta("k", h, s, ce) + v_quanta(h, s, ce)
                        + qk_quanta("q", h, s, ce))

            # ---- out-projection quanta for one strip ----
            def outp_quanta(s):
                quanta = []
                for j in range(4):
                    j_even = (j % 2 == 0)
                    st = {}
                    c_sl = slice(s * 512 + j * 128, s * 512 + (j + 1) * 128)
                    r0 = s * 512 + j * 128

                    def mms(st=st, c_sl=c_sl):
                        st["ps"] = []
                        for g in range(2):
                            ps = op_ps.tile([128, 512], F32, tag="op",
                                            name="opj_ps")
                            for h in range(HEADS_PER_CORE):
                                mm(ps[:], atn[h][:, c_sl],
                                   wo_sb[h][:, g * 512:(g + 1) * 512],
                                   start=(h == 0), stop=(h == 1),
                                   skip_group_check=True)
                            st["ps"].append(ps)

                    def fin(st=st, r0=r0, j_even=j_even):
                        o_sb = outp.tile([128, D], BF16, tag="osb",
                                         name="o_sb")
                        # 2 of 8 PSUM->SBUF copies per strip go to ScE so
                        # DVE isn't the out-proj phase bottleneck
                        if j_even:
                            nc.scalar.copy(o_sb[:, 0:512], st["ps"][0][:])
                        else:
                            nc.vector.tensor_copy(o_sb[:, 0:512],
                                                  st["ps"][0][:])
                        nc.vector.tensor_copy(o_sb[:, 512:1024],
                                              st["ps"][1][:])
                        nc.sync.dma_start(out=out[r0:r0 + 128, :],
                                          in_=o_sb[:])

                    quanta.append((900, mms))
                    quanta.append((0, fin))
                return quanta

            # ---- LoRA quanta ----
            lora_st = {}

            def lora_down(d0, d1):
                if d0 == 0:
                    lora_st["dn_ps"] = op_ps.tile([128, ROWS_PER_CORE], F32,
                                                  tag="op", name="dn_ps")
                for d in range(d0, d1):
                    mm(lora_st["dn_ps"][0:RANK, :],
                       ld_sb[:, d * RANK:(d + 1) * RANK],
                       ct_sb[:, d * ROWS_PER_CORE:(d + 1) * ROWS_PER_CORE],
                       start=(d == 0), stop=(d == 7),
                       skip_group_check=True)

            def lora_dn_copy():
                dn_sb = smallp.tile([RANK, ROWS_PER_CORE], BF16, tag="dn",
                                    name="dn_sb", bufs=1)
                nc.vector.tensor_copy(dn_sb[:], lora_st["dn_ps"][0:RANK, :])
                lora_st["dn_sb"] = dn_sb
                lora_st["lo_sb"] = outp.tile([128, 4 * D], BF16, tag="lo",
                                             bufs=1, name="lo_sb")

            def lora_up(j):
                for g in range(2):
                    up_ps = op_ps.tile([128, 512], F32, tag="op",
                                       name="up_ps")
                    mm(up_ps[:],
                       lora_st["dn_sb"][:, j * 128:(j + 1) * 128],
                       lu_sb[:, g * 512:(g + 1) * 512],
                       start=True, stop=True, skip_group_check=True)
                    nc.vector.tensor_copy(
                        lora_st["lo_sb"][:, j * D + g * 512:
                                         j * D + (g + 1) * 512],
                        up_ps[:])

            def lora_dma():
                nc.sync.dma_start(out=lora_out[:], in_=lora_st["lo_sb"][:])

            lora_quanta = (
                [(900, lambda: lora_down(0, 4)),
                 (900, lambda: lora_down(4, 8)),
                 (0, lora_dn_copy)]
                + [(500, lambda j=j: lora_up(j)) for j in range(4)]
                + [(0, lora_dma)])

            # ---- attention phase (scores + exp + PV + pair-sum tree),
            #      with fill quanta interleaved between matmul groups ----
            add = nc.vector.tensor_add

            def attn_phase(h, s, fill):
                q_sl = slice(s * 512, (s + 1) * 512)
                at = at_ps.tile([HD, 512], F32, tag="at", name="at")
                r = rp.tile([128, 1024], BF16, tag="r", name="r")
                prev_es = None
                for p in range(8):
                    scp = sc_ps.tile([128, 1024], F32, tag="sc", name="scp")
                    for i in range(2):
                        ktile = 2 * p + i
                        mm(scp[:, i * 512:(i + 1) * 512],
                           kt[h][:, ktile * 128:(ktile + 1) * 128],
                           qt[h][:, q_sl],
                           start=True, stop=True, skip_group_check=True)
                    if p == 0:
                        fill.pull(1500)
                    es_p = esp.tile([128, 1024], BF16, tag="es",
                                    name="es_p")
                    nc.scalar.activation(
                        es_p[:], scp[:], mybir.ActivationFunctionType.Exp,
                        scale=float(INV_SQRT_HD))
                    for i in range(2):
                        ktile = 2 * p + i
                        mm(at[:],
                           v_sb[h][:, ktile * 128:(ktile + 1) * 128],
                           es_p[:, i * 512:(i + 1) * 512],
                           start=(ktile == 0), stop=(ktile == NKT - 1),
                           skip_group_check=True)
                    if p % 2 == 1:
                        if p == 1:
                            add(r[:], prev_es[:], es_p[:])
                        else:
                            t = pp.tile([128, 1024], BF16, tag="pp",
                                        name="tpair")
                            # p3/p5 pair-sums ride the otherwise-idle
                            # GpSimd so DVE keeps up in the out-proj
                            # phases; p7 stays on DVE (shortest tail).
                            if GP_PAIR and p in (3, 5):
                                nc.gpsimd.tensor_add(t[:], prev_es[:],
                                                     es_p[:])
                            else:
                                add(t[:], prev_es[:], es_p[:])
                            add(r[:], r[:], t[:])
                    prev_es = es_p
                    if p < 7:
                        fill.pull(900)
                spf = smallp.tile([128, 512], BF16, tag="sp2", name="spf")
                add(spf[:], r[:, 0:512], r[:, 512:1024])

                def finish(split_mul=False):
                    den = op_ps.tile([128, 512], F32, tag="op",
                                     name="den_bc")
                    mm(den[:], ones_sq[:], spf[:],
                       start=True, stop=True, skip_group_check=True)
                    rc = smallp.tile([128, 512], F32, tag="rc", name="rc")
                    nc.vector.reciprocal_approx_fast(rc[:], den[:])
                    if split_mul:
                        # quarter-strip normalize so each out-proj j-tile
                        # of the final strip unblocks as early as possible
                        for j in range(4):
                            jj = slice(j * 128, (j + 1) * 128)
                            nc.vector.tensor_mul(
                                atn[h][:, s * 512 + j * 128:
                                       s * 512 + (j + 1) * 128],
                                at[:, jj], rc[:, jj])
                    else:
                        nc.vector.tensor_mul(atn[h][:, q_sl],
                                             at[:], rc[:])
                return finish

            # ---- schedule ----
            fill = Fill()
            ce_s = nc.scalar.copy
            ce_v = nc.vector.tensor_copy

            # pre-phase: h0 QKV, strip-availability order, copies on ScE
            pre = []
            pre += qk_quanta("k", 0, 0, ce_s)
            pre += qk_quanta("q", 0, 0, ce_s)
            pre += v_quanta(0, 0, ce_s)
            for s in range(1, NSTRIP):
                pre += qk_quanta("k", 0, s, ce_s)
                pre += v_quanta(0, s, ce_s)
            for _, fn in pre:
                fn()

            # A phases: attn(h0, s) filled with remaining h0 q-projs,
            # h1 qkv, lora (ScE/DVE have slack here), deferred finishes.
            for q in [0, 1, 2]:
                for c, fn in qk_quanta("q", 0, 1 + q, ce_v):
                    fill.add(c, fn)
            fin = {}
            fin[(0, 0)] = attn_phase(0, 0, fill)

            a_extra = {1: lora_quanta[0:3], 2: lora_quanta[3:5],
                       3: lora_quanta[5:]}
            for s in range(1, NSTRIP):
                for c, fn in qkv_quanta(1, s - 1, ce_v):
                    fill.add(c, fn)
                fill.add(250, fin[(0, s - 1)])
                for c, fn in a_extra[s]:
                    fill.add(c, fn)
                fin[(0, s)] = attn_phase(0, s, fill)

            # B phases: attn(h1, s) filled with last h1 qkv,
            # out-projections, and deferred finishes.
            for c, fn in qkv_quanta(1, 3, ce_v):
                fill.add(c, fn)
            fill.add(250, fin[(0, 3)])
            fin[(1, 0)] = attn_phase(1, 0, fill)

            op_q = {s: outp_quanta(s) for s in range(NSTRIP)}
            for s in range(3):
                fill.add(250, fin[(1, s)])
                for c, fn in op_q[s]:
                    fill.add(c, fn)
                fin[(1, s + 1)] = attn_phase(1, s + 1, fill)

            fill.add(250, lambda: fin[(1, 3)](split_mul=True))
            for c, fn in op_q[3]:
                fill.add(c, fn)
            fill.drain()

    nc.compile()
    _CACHE["nc"] = nc
    return nc


def _prep_in_maps(inputs):
    bf = ml_dtypes.bfloat16
    hidden = np.asarray(inputs["hidden_states"], dtype=np.float32)
    control = np.asarray(inputs["control_states"], dtype=np.float32)
    Wq = np.asarray(inputs["Wq"], dtype=np.float32)
    Wk = np.asarray(inputs["Wk"], dtype=np.float32)
    Wv = np.asarray(inputs["Wv"], dtype=np.float32)
    Wo = np.asarray(inputs["Wo"], dtype=np.float32)
    ldT = np.ascontiguousarray(
        np.asarray(inputs["lora_down"], dtype=np.float32).T.astype(bf)
        .reshape(8, 128, RANK).transpose(1, 0, 2).reshape(128, 8 * RANK))
    luT = np.ascontiguousarray(
        np.asarray(inputs["lora_up"], dtype=np.float32).T.astype(bf))

    def wshuf(wT):  # [1024, 256] -> [2*128, 8*128] (head-major rows)
        return np.ascontiguousarray(
            wT.reshape(8, 128, 2, HD).transpose(2, 1, 0, 3)
            .reshape(2 * 128, 8 * HD))

    xsh_b = []
    for b in range(B):
        xT = hidden[b].T.astype(bf)  # [1024, 2048]
        xsh_b.append(np.ascontiguousarray(
            xT.reshape(8, 128, NSTRIP, 512).transpose(2, 1, 0, 3)
            .reshape(NSTRIP * 128, 8 * 512)))
    cT_full = control.reshape(SG, D).T.astype(bf)

    in_maps = []
    for c in range(N_CORES):
        b = c // 4
        g = c % 4
        hs = slice(g * 256, (g + 1) * 256)
        rs = slice(c * ROWS_PER_CORE, (c + 1) * ROWS_PER_CORE)
        ct = cT_full[:, rs]  # [1024, 512]
        in_maps.append({
            "xsh": xsh_b[b],
            "wqT": wshuf(Wq[hs, :].T.astype(bf)),
            "wkT": wshuf(Wk[hs, :].T.astype(bf)),
            "wvT": wshuf(Wv[hs, :].T.astype(bf)),
            "woT": np.ascontiguousarray(Wo[:, hs].T.astype(bf)),
            "cT": np.ascontiguousarray(
                ct.reshape(8, 128, ROWS_PER_CORE).transpose(1, 0, 2)
                .reshape(128, 8 * ROWS_PER_CORE)),
            "ldT": ldT,
            "luT": luT,
        })
    return in_maps


def _reduce_outputs(results, bo):
    total = np.zeros((B, S, D), dtype=np.float32)
    for c in range(N_CORES):
        b = c // 4
        total[b] += results[c]["out"].astype(np.float32)
    total += np.asarray(bo, dtype=np.float32)[None, None, :]
    flat = total.reshape(SG, D)
    for c in range(N_CORES):
        rs = slice(c * ROWS_PER_CORE, (c + 1) * ROWS_PER_CORE)
        lo = results[c]["lora_out"].astype(np.float32)
        flat[rs] += lo.reshape(128, 4, D).transpose(1, 0, 2).reshape(
            ROWS_PER_CORE, D)
    return flat.reshape(B, S, D)


def kernel(**inputs):
    nc = build_program()
    in_maps = _prep_in_maps(inputs)
    res = run_bass_kernel_spmd(nc, in_maps, list(range(N_CORES)))
    return _reduce_outputs(res.results, inputs["bo"])


# revision 38
# speedup vs baseline: 1.0261x; 1.0118x over previous
"""Trainium2 Bass kernel for ControlLoRACrossAttnProcessor (final).

Batch x head-group sharding over 8 NeuronCores: core c owns batch c//4
and heads 2*(c%4), 2*(c%4)+1.  Each core computes both heads' full
attention over its batch and a partial output projection (contraction
over its 256 Wo rows); the host sums 4 partials per batch and adds bo.
All matmuls bf16 (PSUM fp32); measured ~153-155us vs the 174.7us
baseline, rel err 3.9e-3.

Schedule design (why it is fast):
- MM-granularity interleaving: independent projection / out-proj /
  LoRA matmuls are emitted *between* the score/PV matmul pairs of each
  attention phase via a fill queue, so the PE always has queued work
  while PV waits on ScalarE's exp (the per-p-step chain limiter).
- The softmax-denominator finish (ones-matmul + reciprocal + normalize)
  for (h, s) is deferred into the NEXT phase's fill stream, so the PE
  never stalls on the DVE pair-sum tree; the final strip normalizes in
  quarter-strips so its out-proj tiles unblock early.
- HAM warmup: two short *accumulating* matmul chains run back-to-back
  during the ~8us fixed DMA/preamble lead-in, flipping the PE clock
  gate to 2.4GHz before real data lands (spaced dummies never do --
  isolated MMs pace at ~0.5-1.2us and never fill a 3.4us busy window).
- Startup: first strip DMA'd in 8 chunks behind the first weights, so
  k/q projections start per-chunk; exp table preloaded with a dummy
  activation during the lead-in.
- Engine balance: pre-phase PSUM copies on ScE (idle before exp work
  starts), mid-kernel copies on DVE, 2 of 8 out-proj copies per strip
  on ScE; LoRA rides the A-phases where ScE/DVE have slack.

Measured dead ends (do not revisit without new evidence): GpSimd
tensor_add is 2.6us per [128,1024] tile; splitting exp into [128,512]
halves costs +257ns/pair in fixed per-op overhead; fp8 DoubleRow
out-projection hung the exec unit; fp8 on any two of {qkv, pv, out}
exceeds the 2e-2 error gate (each alone is 1.4-1.9e-2).
"""

import sys

for _p in ("/opt/trn_rl_repo", "/root/.axon_site"):
    if _p not in sys.path:
        sys.path.insert(0, _p)

import numpy as np
import ml_dtypes

import concourse.bass as bass  # noqa: E402
import concourse.mybir as mybir  # noqa: E402
import concourse.bass_isa as bass_isa  # noqa: E402
from concourse import bacc  # noqa: E402
from concourse.bass_utils import run_bass_kernel_spmd  # noqa: E402
from concourse.tile import TileContext  # noqa: E402
from concourse.masks import make_identity  # noqa: E402

dt = mybir.dt

B, S, D = 2, 2048, 1024
H = 8
HD = 128
RANK = 4
N_CORES = 8
SG = B * S
ROWS_PER_CORE = SG // N_CORES      # 512 lora rows per core
NSTRIP = S // 512                  # 4 query strips of 512
NKT = S // 128                     # 16 key tiles of 128
HEADS_PER_CORE = 2
INV_SQRT_HD = 1.0 / np.sqrt(np.float32(HD))

F32 = dt.float32
BF16 = dt.bfloat16
F8 = dt.float8e4

# fp8 out-projection scales: atn is written as 32*(softmax-avg of v)
# (bounded by 32*max|v| ~ 115 < 240), Wo is host-scaled by 1024; the
# host divides the partial sums by 32*1024 when reducing.
S_ATN = 32.0
S_WO = 1024.0

# offload the p3/p5 softmax pair-sums to GpSimd (DVE relief in the
# out-projection phases).  Measured 2.6us per [128,1024] add on HW --
# slower than DVE and it stalls the accumulate chain, so OFF.
GP_PAIR = False

_CACHE = {}


class Fill:
    """FIFO of (pe_cost_ns, emit_fn) quanta pulled between attention
    matmul groups so the PE queue never starves while PV waits on exp."""

    def __init__(self):
        self.q = []

    def add(self, cost, fn):
        self.q.append((cost, fn))

    def pull(self, budget):
        spent = 0
        while self.q and spent < budget:
            c, fn = self.q.pop(0)
            fn()
            spent += c

    def drain(self):
        while self.q:
            _, fn = self.q.pop(0)
            fn()


def build_program():
    if "nc" in _CACHE:
        return _CACHE["nc"]

    nc = bacc.Bacc("TRN2", target_bir_lowering=False, debug=False,
                   num_devices=N_CORES)

    # Pre-shuffled host layouts: every DMA is a plain contiguous 2D copy.
    xsh = nc.declare_dram_parameter("xsh", [NSTRIP * 128, 8 * 512], BF16,
                                    isOutput=False)
    wqT = nc.declare_dram_parameter("wqT", [2 * 128, 8 * HD], BF16,
                                    isOutput=False)
    wkT = nc.declare_dram_parameter("wkT", [2 * 128, 8 * HD], BF16,
                                    isOutput=False)
    wvT = nc.declare_dram_parameter("wvT", [2 * 128, 8 * HD], BF16,
                                    isOutput=False)
    woT = nc.declare_dram_parameter("woT", [256, D], BF16, isOutput=False)
    cT = nc.declare_dram_parameter("cT", [128, 8 * ROWS_PER_CORE], BF16,
                                   isOutput=False)
    ldT = nc.declare_dram_parameter("ldT", [128, 8 * RANK], BF16,
                                    isOutput=False)
    luT = nc.declare_dram_parameter("luT", [RANK, D], BF16, isOutput=False)
    out = nc.declare_dram_parameter("out", [S, D], BF16, isOutput=True)
    lora_out = nc.declare_dram_parameter("lora_out", [128, 4 * D],
                                         BF16, isOutput=True)

    with TileContext(nc) as tc:
        with tc.tile_pool(name="const", bufs=1) as constp, \
             tc.tile_pool(name="wts", bufs=1) as wts, \
             tc.tile_pool(name="xs", bufs=1) as xsp, \
             tc.tile_pool(name="qkv", bufs=1) as qkvp, \
             tc.tile_pool(name="stage", bufs=2) as stagep, \
             tc.tile_pool(name="es", bufs=6) as esp, \
             tc.tile_pool(name="rp", bufs=2) as rp, \
             tc.tile_pool(name="pp", bufs=2) as pp, \
             tc.tile_pool(name="small", bufs=2) as smallp, \
             tc.tile_pool(name="outp", bufs=2) as outp, \
             tc.tile_pool(name="sc_ps", bufs=2, space="PSUM") as sc_ps, \
             tc.tile_pool(name="at_ps", bufs=2, space="PSUM") as at_ps, \
             tc.tile_pool(name="op_ps", bufs=2, space="PSUM") as op_ps:

            mm = nc.tensor.matmul

            # ---- constants ----
            ident = constp.tile([128, 128], BF16, tag="ident")
            make_identity(nc, ident[:])
            dummy = constp.tile([128, 512], BF16, tag="dummy")
            nc.vector.memset(dummy[:], 0.0)
            ones_sq = constp.tile([128, 128], BF16, tag="ones_sq")
            nc.vector.memset(ones_sq[:], 1.0)

            # ---- DMA loads, ordered for earliest compute start ----
            ld_sb = wts.tile([128, 8 * RANK], BF16, tag="ld")
            lu_sb = wts.tile([RANK, D], BF16, tag="lu")
            ct_sb = wts.tile([128, 8 * ROWS_PER_CORE], BF16, tag="ct")

            w_sb = {}
            for h in range(HEADS_PER_CORE):
                for nm in ("q", "k", "v"):
                    w_sb[(nm, h)] = wts.tile([128, 8 * HD], BF16,
                                             tag=f"w{nm}{h}", name=f"w{nm}{h}")
            wo_sb = [wts.tile([HD, D], BF16, tag=f"wo{h}", name=f"wo{h}")
                     for h in range(HEADS_PER_CORE)]
            xs = [xsp.tile([128, 8 * 512], BF16, tag=f"xs{s}", name=f"xs{s}")
                  for s in range(NSTRIP)]

            wsrc = {"q": wqT, "k": wkT, "v": wvT}

            def load_w(nm, h):
                nc.sync.dma_start(out=w_sb[(nm, h)][:],
                                  in_=wsrc[nm][h * 128:(h + 1) * 128, :])

            # first strip in 4 chunk-pair pieces so k/q projections can
            # start on chunk 0; later strips in halves.
            load_w("k", 0)
            load_w("q", 0)
            for ch in range(8):
                nc.sync.dma_start(
                    out=xs[0][:, ch * 512:(ch + 1) * 512],
                    in_=xsh[0:128, ch * 512:(ch + 1) * 512])
            load_w("v", 0)
            nc.sync.dma_start(out=ld_sb[:], in_=ldT[:])
            nc.sync.dma_start(out=lu_sb[:], in_=luT[:])
            for s in range(1, NSTRIP):
                for half in range(2):
                    nc.sync.dma_start(
                        out=xs[s][:, half * 2048:(half + 1) * 2048],
                        in_=xsh[s * 128:(s + 1) * 128,
                                half * 2048:(half + 1) * 2048])
            for nm in ("k", "q", "v"):
                load_w(nm, 1)
            for h in range(HEADS_PER_CORE):
                nc.sync.dma_start(out=wo_sb[h][:],
                                  in_=woT[h * HD:(h + 1) * HD, :])
            nc.sync.dma_start(out=ct_sb[:], in_=cT[:])

            # ---- PE warmup + ACT exp-table preload during DMA lead-in.
            # Accumulating matmul chains have no inter-MM dependencies, so
            # they run back-to-back: a contiguous >3.4us busy window that
            # flips the HAM clock gate to 2.4GHz before real data lands.
            for grp, n_mm in ((0, 8), (1, 4)):
                wu_ps = op_ps.tile([128, 512], F32, tag="op", name="wu_ps")
                for i in range(n_mm):
                    mm(wu_ps[:], dummy[:, 0:128], dummy[:],
                       start=(i == 0), stop=(i == n_mm - 1))
            exp_warm = smallp.tile([128, 16], BF16, tag="spf",
                                   name="exp_warm")
            nc.scalar.activation(exp_warm[:], dummy[:, 0:16],
                                 mybir.ActivationFunctionType.Exp)

            # ---- persistent SBUF state ----
            qt = [qkvp.tile([HD, S], BF16, tag=f"qt{h}", name=f"qt{h}")
                  for h in range(HEADS_PER_CORE)]
            kt = [qkvp.tile([HD, S], BF16, tag=f"kt{h}", name=f"kt{h}")
                  for h in range(HEADS_PER_CORE)]
            v_sb = [qkvp.tile([128, S], BF16, tag=f"v{h}", name=f"v{h}")
                    for h in range(HEADS_PER_CORE)]
            atn = [qkvp.tile([HD, S], BF16, tag=f"atn{h}", name=f"atn{h}")
                   for h in range(HEADS_PER_CORE)]

            # ---- projection emitters ----
            def proj_mms(st, nm, h, s, d0, d1):
                if d0 == 0:
                    st["ps"] = op_ps.tile([128, 512], F32, tag="op",
                                          name=f"pj_{nm}{h}{s}")
                for d in range(d0, d1):
                    mm(st["ps"][:],
                       w_sb[(nm, h)][:, d * HD:(d + 1) * HD],
                       xs[s][:, d * 512:(d + 1) * 512],
                       start=(d == 0), stop=(d == 7),
                       skip_group_check=True)

            def qk_quanta(nm, h, s, ce):
                st = {}
                dst = qt[h] if nm == "q" else kt[h]
                sl = slice(s * 512, (s + 1) * 512)
                return [
                    (900, lambda: proj_mms(st, nm, h, s, 0, 4)),
                    (900, lambda: proj_mms(st, nm, h, s, 4, 8)),
                    (0, lambda: ce(dst[:, sl], st["ps"][:])),
                ]

            def v_quanta(h, s, ce):
                st = {}
                sl = slice(s * 512, (s + 1) * 512)

                def stage_copy():
                    st["vt"] = stagep.tile([HD, 512], BF16, tag="vst",
                                           name="vt_stage")
                    ce(st["vt"][:], st["ps"][:])

                def transp():
                    st["tq"] = op_ps.tile([128, 512], BF16, tag="op",
                                          name="tq")
                    for i in range(4):
                        nc.tensor.transpose(
                            st["tq"][:, i * 128:(i + 1) * 128],
                            st["vt"][:, i * 128:(i + 1) * 128], ident[:])

                return [
                    (900, lambda: proj_mms(st, "v", h, s, 0, 4)),
                    (900, lambda: proj_mms(st, "v", h, s, 4, 8)),
                    (0, stage_copy),
                    (500, transp),
                    (0, lambda: ce(v_sb[h][:, sl], st["tq"][:])),
                ]

            def qkv_quanta(h, s, ce):
                return (qk_quanta("k", h, s, ce) + v_quanta(h, s, ce)
                        + qk_quanta("q", h, s, ce))

            # ---- out-projection quanta for one strip ----
            def outp_quanta(s):
                quanta = []
                for j in range(4):
                    j_even = (j % 2 == 0)
                    st = {}
                    c_sl = slice(s * 512 + j * 128, s * 512 + (j + 1) * 128)
                    r0 = s * 512 + j * 128

                    def mms(st=st, c_sl=c_sl):
                        st["ps"] = []
                        for g in range(2):
                            ps = op_ps.tile([128, 512], F32, tag="op",
                                            name="opj_ps")
                            for h in range(HEADS_PER_CORE):
                                mm(ps[:], atn[h][:, c_sl],
                                   wo_sb[h][:, g * 512:(g + 1) * 512],
                                   start=(h == 0), stop=(h == 1),
                                   skip_group_check=True)
                            st["ps"].append(ps)

                    def fin(st=st, r0=r0, j_even=j_even):
                        o_sb = outp.tile([128, D], BF16, tag="osb",
                                         name="o_sb")
                        # 2 of 8 PSUM->SBUF copies per strip go to ScE so
                        # DVE isn't the out-proj phase bottleneck
                        if j_even:
                            nc.scalar.copy(o_sb[:, 0:512], st["ps"][0][:])
                        else:
                            nc.vector.tensor_copy(o_sb[:, 0:512],
                                                  st["ps"][0][:])
                        nc.vector.tensor_copy(o_sb[:, 512:1024],
                                              st["ps"][1][:])
                        nc.sync.dma_start(out=out[r0:r0 + 128, :],
                                          in_=o_sb[:])

                    quanta.append((900, mms))
                    quanta.append((0, fin))
                return quanta

            # ---- LoRA quanta ----
            lora_st = {}

            def lora_down(d0, d1):
                if d0 == 0:
                    lora_st["dn_ps"] = op_ps.tile([128, ROWS_PER_CORE], F32,
                                                  tag="op", name="dn_ps")
                for d in range(d0, d1):
                    mm(lora_st["dn_ps"][0:RANK, :],
                       ld_sb[:, d * RANK:(d + 1) * RANK],
                       ct_sb[:, d * ROWS_PER_CORE:(d + 1) * ROWS_PER_CORE],
                       start=(d == 0), stop=(d == 7),
                       skip_group_check=True)

            def lora_dn_copy():
                dn_sb = smallp.tile([RANK, ROWS_PER_CORE], BF16, tag="dn",
                                    name="dn_sb", bufs=1)
                nc.vector.tensor_copy(dn_sb[:], lora_st["dn_ps"][0:RANK, :])
                lora_st["dn_sb"] = dn_sb
                lora_st["lo_sb"] = outp.tile([128, 4 * D], BF16, tag="lo",
                                             bufs=1, name="lo_sb")

            def lora_up(j):
                for g in range(2):
                    up_ps = op_ps.tile([128, 512], F32, tag="op",
                                       name="up_ps")
                    mm(up_ps[:],
                       lora_st["dn_sb"][:, j * 128:(j + 1) * 128],
                       lu_sb[:, g * 512:(g + 1) * 512],
                       start=True, stop=True, skip_group_check=True)
                    nc.vector.tensor_copy(
                        lora_st["lo_sb"][:, j * D + g * 512:
                                         j * D + (g + 1) * 512],
                        up_ps[:])

            def lora_dma():
                nc.sync.dma_start(out=lora_out[:], in_=lora_st["lo_sb"][:])

            lora_quanta = (
                [(900, lambda: lora_down(0, 4)),
                 (900, lambda: lora_down(4, 8)),
                 (0, lora_dn_copy)]
                + [(500, lambda j=j: lora_up(j)) for j in range(4)]
                + [(0, lora_dma)])

            # ---- attention phase (scores + exp + PV + pair-sum tree),
            #      with fill quanta interleaved between matmul groups ----
            add = nc.vector.tensor_add

            def attn_phase(h, s, fill):
                q_sl = slice(s * 512, (s + 1) * 512)
                at = at_ps.tile([HD, 512], F32, tag="at", name="at")
                r = rp.tile([128, 1024], BF16, tag="r", name="r")
                prev_es = None
                for p in range(8):
                    scp = sc_ps.tile([128, 1024], F32, tag="sc", name="scp")
                    for i in range(2):
                        ktile = 2 * p + i
                        mm(scp[:, i * 512:(i + 1) * 512],
                           kt[h][:, ktile * 128:(ktile + 1) * 128],
                           qt[h][:, q_sl],
                           start=True, stop=True, skip_group_check=True)
                    if p == 0:
                        fill.pull(1500)
                    es_p = esp.tile([128, 1024], BF16, tag="es",
                                    name="es_p")
                    nc.scalar.activation(
                        es_p[:], scp[:], mybir.ActivationFunctionType.Exp,
                        scale=float(INV_SQRT_HD))
                    for i in range(2):
                        ktile = 2 * p + i
                        mm(at[:],
                           v_sb[h][:, ktile * 128:(ktile + 1) * 128],
                           es_p[:, i * 512:(i + 1) * 512],
                           start=(ktile == 0), stop=(ktile == NKT - 1),
                           skip_group_check=True)
                    if p % 2 == 1:
                        if p == 1:
                            add(r[:], prev_es[:], es_p[:])
                        else:
                            t = pp.tile([128, 1024], BF16, tag="pp",
                                        name="tpair")
                            # p3/p5 pair-sums ride the otherwise-idle
                            # GpSimd so DVE keeps up in the out-proj
                            # phases; p7 stays on DVE (shortest tail).
                            if GP_PAIR and p in (3, 5):
                                nc.gpsimd.tensor_add(t[:], prev_es[:],
                                                     es_p[:])
                            else:
                                add(t[:], prev_es[:], es_p[:])
                            add(r[:], r[:], t[:])
                    prev_es = es_p
                    if p < 7:
                        fill.pull(900)
                spf = smallp.tile([128, 512], BF16, tag="sp2", name="spf")
                add(spf[:], r[:, 0:512], r[:, 512:1024])

                def finish(split_mul=False):
                    den = op_ps.tile([128, 512], F32, tag="op",
                                     name="den_bc")
                    mm(den[:], ones_sq[:], spf[:],
                       start=True, stop=True, skip_group_check=True)
                    rc = smallp.tile([128, 512], F32, tag="rc", name="rc")
                    nc.vector.reciprocal_approx_fast(rc[:], den[:])
                    if split_mul:
                        # quarter-strip normalize so each out-proj j-tile
                        # of the final strip unblocks as early as possible
                        for j in range(4):
                            jj = slice(j * 128, (j + 1) * 128)
                            nc.vector.tensor_mul(
                                atn[h][:, s * 512 + j * 128:
                                       s * 512 + (j + 1) * 128],
                                at[:, jj], rc[:, jj])
                    else:
                        nc.vector.tensor_mul(atn[h][:, q_sl],
                                             at[:], rc[:])
                return finish

            # ---- schedule ----
            fill = Fill()
            ce_s = nc.scalar.copy
            ce_v = nc.vector.tensor_copy

            # pre-phase: h0 QKV, strip-availability order, copies on ScE
            pre = []
            pre += qk_quanta("k", 0, 0, ce_s)
            pre += qk_quanta("q", 0, 0, ce_s)
            pre += v_quanta(0, 0, ce_s)
            for s in range(1, NSTRIP):
                pre += qk_quanta("k", 0, s, ce_s)
                pre += v_quanta(0, s, ce_s)
            for _, fn in pre:
                fn()

            # A phases: attn(h0, s) filled with remaining h0 q-projs,
            # h1 qkv, lora (ScE/DVE have slack here), deferred finishes.
            for q in [0, 1, 2]:
                for c, fn in qk_quanta("q", 0, 1 + q, ce_v):
                    fill.add(c, fn)
            fin = {}
            fin[(0, 0)] = attn_phase(0, 0, fill)

            a_extra = {1: lora_quanta[0:3], 2: lora_quanta[3:5],
                       3: lora_quanta[5:]}
            for s in range(1, NSTRIP):
                for c, fn in qkv_quanta(1, s - 1, ce_v):
                    fill.add(c, fn)
                fill.add(250, fin[(0, s - 1)])
                for c, fn in a_extra[s]:
                    fill.add(c, fn)
                fin[(0, s)] = attn_phase(0, s, fill)

            # B phases: attn(h1, s) filled with last h1 qkv,
            # out-projections, and deferred finishes.
            for c, fn in qkv_quanta(1, 3, ce_v):
                fill.add(c, fn)
            fill.add(250, fin[(0, 3)])
            fin[(1, 0)] = attn_phase(1, 0, fill)

            op_q = {s: outp_quanta(s) for s in range(NSTRIP)}
            for s in range(3):
                fill.add(250, fin[(1, s)])
                for c, fn in op_q[s]:
                    fill.add(c, fn)
                fin[(1, s + 1)] = attn_phase(1, s + 1, fill)

            fill.add(250, lambda: fin[(1, 3)](split_mul=True))
            for c, fn in op_q[3]:
                fill.add(c, fn)
            fill.drain()

    nc.compile()
    _CACHE["nc"] = nc
    return nc


def _prep_in_maps(inputs):
    bf = ml_dtypes.bfloat16
    hidden = np.asarray(inputs["hidden_states"], dtype=np.float32)
    control = np.asarray(inputs["control_states"], dtype=np.float32)
    Wq = np.asarray(inputs["Wq"], dtype=np.float32)
    Wk = np.asarray(inputs["Wk"], dtype=np.float32)
    Wv = np.asarray(inputs["Wv"], dtype=np.float32)
    Wo = np.asarray(inputs["Wo"], dtype=np.float32)
    ldT = np.ascontiguousarray(
        np.asarray(inputs["lora_down"], dtype=np.float32).T.astype(bf)
        .reshape(8, 128, RANK).transpose(1, 0, 2).reshape(128, 8 * RANK))
    luT = np.ascontiguousarray(
        np.asarray(inputs["lora_up"], dtype=np.float32).T.astype(bf))

    def wshuf(wT):  # [1024, 256] -> [2*128, 8*128] (head-major rows)
        return np.ascontiguousarray(
            wT.reshape(8, 128, 2, HD).transpose(2, 1, 0, 3)
            .reshape(2 * 128, 8 * HD))

    xsh_b = []
    for b in range(B):
        xT = hidden[b].T.astype(bf)  # [1024, 2048]
        xsh_b.append(np.ascontiguousarray(
            xT.reshape(8, 128, NSTRIP, 512).transpose(2, 1, 0, 3)
            .reshape(NSTRIP * 128, 8 * 512)))
    cT_full = control.reshape(SG, D).T.astype(bf)

    in_maps = []
    for c in range(N_CORES):
        b = c // 4
        g = c % 4
        hs = slice(g * 256, (g + 1) * 256)
        rs = slice(c * ROWS_PER_CORE, (c + 1) * ROWS_PER_CORE)
        ct = cT_full[:, rs]  # [1024, 512]
        in_maps.append({
            "xsh": xsh_b[b],
            "wqT": wshuf(Wq[hs, :].T.astype(bf)),
            "wkT": wshuf(Wk[hs, :].T.astype(bf)),
            "wvT": wshuf(Wv[hs, :].T.astype(bf)),
            "woT": np.ascontiguousarray(Wo[:, hs].T.astype(bf)),
            "cT": np.ascontiguousarray(
                ct.reshape(8, 128, ROWS_PER_CORE).transpose(1, 0, 2)
                .reshape(128, 8 * ROWS_PER_CORE)),
            "ldT": ldT,
            "luT": luT,
        })
    return in_maps


def _reduce_outputs(results, bo):
    total = np.zeros((B, S, D), dtype=np.float32)
    for c in range(N_CORES):
        b = c // 4
        total[b] += results[c]["out"].astype(np.float32)
    total += np.asarray(bo, dtype=np.float32)[None, None, :]
    flat = total.reshape(SG, D)
    for c in range(N_CORES):
        rs = slice(c * ROWS_PER_CORE, (c + 1) * ROWS_PER_CORE)
        lo = results[c]["lora_out"].astype(np.float32)
        flat[rs] += lo.reshape(128, 4, D).transpose(1, 0, 2).reshape(
            ROWS_PER_CORE, D)
    return flat.reshape(B, S, D)


def kernel(**inputs):
    nc = build_program()
    in_maps = _prep_in_maps(inputs)
    res = run_bass_kernel_spmd(nc, in_maps, list(range(N_CORES)))
    return _reduce_outputs(res.results, inputs["bo"])
